# revision 29
# baseline (speedup 1.0000x reference)
"""Multi-head attention forward (B=2, S=2048, D=1024, H=16) on 8 Trainium2
NeuronCores, tensor-parallel over heads (2 heads per core).

v3: fp16 datapath + XBAR DMA transpose for V.
  - All matmul operands fp16 (same 1 cyc/row PE stream rate as f32r, half the
    SBUF/PSUM/DMA traffic, 2x DVE modes); PSUM accumulation stays fp32.
  - qT/kT/vT projections from a host-pretransposed fp16 xT; bias add on DVE
    while copying PSUM->SBUF fp16.
  - V transposed into the per-k-tile augmented operand via the DMA XBAR
    (dma_start transpose=True on the SP HWDGE queue) instead of PE matmul
    transposes - frees PE cycles and the DVE aug-fill copies.
  - scoresT[k, q] = kT_h.T @ qT_h per (batch, head); exp via ScalarE with the
    1/sqrt(64) folded into the activation scale, probs written as fp16.
  - ctxT accumulated over k-tiles with an augmented V (ones column) so the
    softmax denominators fall out of the same matmuls for free.
  - normalize: broadcast the sums row with a PE ones-matmul, fast reciprocal
    on DVE, multiply into fp16 ctxT; output projection per 128-token tile,
    fp16 partials DMA'd out; host sums the 8 partial outputs + bo in fp32.
  - loop order: qc outer / head inner, out_proj(1) emitted per qc so the
    final output DMAs overlap the attention tail.
"""
import sys
import os

sys.path.insert(0, '/opt/trn_rl_repo')

import numpy as np
import concourse.bass as bass
import concourse.mybir as mybir
import concourse.tile as tile
from concourse import bacc, bass_utils
import contextlib

f32 = mybir.dt.float32
f16 = mybir.dt.float16
EXP = mybir.ActivationFunctionType.Exp

B, S, D, H, HD = 2, 2048, 1024, 16, 64
T = B * S              # 4096 tokens
DC = 128               # dims per core (2 heads)
KT = 8                 # feature k-tiles (D / 128)
NCH = 8                # projection chunks of 512 tokens
NKT = 16               # k-token tiles per batch (S / 128)
NQC = 4                # q chunks of 512 per (b, h)


def _build():
    nc = bacc.Bacc("TRN2", target_bir_lowering=False, debug=False)
    xT_d = nc.dram_tensor("xT", [D, T], f16, kind="ExternalInput").ap()
    wqT_d = nc.dram_tensor("wqT", [D, DC], f16, kind="ExternalInput").ap()
    wkT_d = nc.dram_tensor("wkT", [D, DC], f16, kind="ExternalInput").ap()
    wvT_d = nc.dram_tensor("wvT", [D, DC], f16, kind="ExternalInput").ap()
    woT_d = nc.dram_tensor("woT", [DC, D], f16, kind="ExternalInput").ap()
    bq_d = nc.dram_tensor("bq", [DC, 1], f32, kind="ExternalInput").ap()
    bk_d = nc.dram_tensor("bk", [DC, 1], f32, kind="ExternalInput").ap()
    bv_d = nc.dram_tensor("bv", [DC, 1], f32, kind="ExternalInput").ap()
    out_d = nc.dram_tensor("out", [T, D], f16, kind="ExternalOutput").ap()

    xT_ap = xT_d.rearrange("(kt p) t -> p kt t", p=128)

    with tile.TileContext(nc) as tc:
        ctx = contextlib.ExitStack()
        cpool = ctx.enter_context(tc.tile_pool(name="cpool", bufs=1))
        xpool = ctx.enter_context(tc.tile_pool(name="xpool", bufs=4))
        ppool = ctx.enter_context(tc.tile_pool(name="ppool", bufs=8))
        npool = ctx.enter_context(tc.tile_pool(name="npool", bufs=2))
        opool = ctx.enter_context(tc.tile_pool(name="opool", bufs=3))
        pj = ctx.enter_context(tc.tile_pool(name="pj", bufs=2, space="PSUM"))
        sc = ctx.enter_context(tc.tile_pool(name="sc", bufs=2, space="PSUM"))
        cx = ctx.enter_context(tc.tile_pool(name="cx", bufs=2, space="PSUM"))

        # ---- constants / persistent tiles ----
        wqr = cpool.tile([128, KT, DC], f16, tag="wqr")
        wkr = cpool.tile([128, KT, DC], f16, tag="wkr")
        wvr = cpool.tile([128, KT, DC], f16, tag="wvr")
        nc.gpsimd.dma_start(wqr[:], wqT_d.rearrange("(kt p) m -> p kt m", p=128))
        nc.gpsimd.dma_start(wkr[:], wkT_d.rearrange("(kt p) m -> p kt m", p=128))
        nc.gpsimd.dma_start(wvr[:], wvT_d.rearrange("(kt p) m -> p kt m", p=128))
        wor = cpool.tile([128, D], f16, tag="wor")
        nc.gpsimd.dma_start(wor[:], woT_d[:])
        bq = cpool.tile([DC, 1], f32, tag="bq")
        bk = cpool.tile([DC, 1], f32, tag="bk")
        bv = cpool.tile([DC, 1], f32, tag="bv")
        nc.sync.dma_start(bq[:], bq_d[:])
        nc.sync.dma_start(bk[:], bk_d[:])
        nc.sync.dma_start(bv[:], bv_d[:])

        ones = cpool.tile([128, 128], f16, tag="ones")
        nc.vector.memset(ones[:], 1.0)

        # aug[p, tt, h, :]: per k-token-tile per head augmented V operand.
        # h0: v dims at cols 0..63, ones col 64 -> ctx rows 0..63, sums row 64
        # h1: v dims at cols 64..127, ones col 0 -> ctx rows 64..127, sums row 0
        aug = cpool.tile([128, B * NKT, 2, 128], f16, tag="aug")
        nc.vector.memset(aug[:], 0.0)
        nc.vector.tensor_copy(aug[:, :, 0, 64:65], ones[:, 0:B * NKT])
        nc.vector.tensor_copy(aug[:, :, 1, 0:1], ones[:, 0:B * NKT])

        qTr = cpool.tile([128, T], f16, tag="qTr")
        kTr = cpool.tile([128, T], f16, tag="kTr")
        vTs = cpool.tile([128, T], f16, tag="vTs")
        ctxT = [cpool.tile([128, S], f16, tag=f"ctxT{b}", name=f"ctxT{b}")
                for b in range(B)]

        # ---- phase 1: projections + v transposes ----
        _xtiles = {}

        def load_chunk(ch):
            csl = slice(ch * 512, (ch + 1) * 512)
            xTr = xpool.tile([128, KT, 512], f16, tag="xTr")
            if ch == 0:
                # split the first chunk's load per feature tile so the first
                # projection matmuls can start as soon as f=0 lands
                for f in range(KT):
                    nc.sync.dma_start(xTr[:, f], xT_ap[:, f, csl])
            else:
                # batch-1 chunks go on the gpsimd SWDGE queue: the SP queue
                # is saturated with XBAR transposes during attention(0)
                eng = nc.sync if ch < 4 else nc.gpsimd
                eng.dma_start(xTr[:], xT_ap[:, :, csl])
            _xtiles[ch] = xTr

        def proj_one(ch, wr, b_t, dst):
            csl = slice(ch * 512, (ch + 1) * 512)
            xTr = _xtiles[ch]
            pp = pj.tile([128, 512], f32, tag="pj")
            for f in range(KT):
                nc.tensor.matmul(pp[:], wr[:, f], xTr[:, f],
                                 start=(f == 0), stop=(f == KT - 1))
            nc.vector.tensor_scalar_add(dst[:, csl], pp[:], b_t[:])

        def transp_chunk(ch):
            # transpose v into the aug tiles via the DMA XBAR: one transfer
            # per head covers this chunk's 4 k-token tiles
            csl = slice(ch * 512, (ch + 1) * 512)
            for h in range(2):
                nc.sync.dma_start(
                    aug[:, ch * 4:(ch + 1) * 4, h, h * 64:(h + 1) * 64],
                    vTs[h * 64:(h + 1) * 64, csl], transpose=True)

        # ---- phase 2/3: attention + output projection ----
        def attention_qc(b, qc, fillers=None):
            qsl = slice(b * S + qc * 512, b * S + (qc + 1) * 512)
            osl = slice(qc * 512, (qc + 1) * 512)
            # the two heads' kp loops run interleaved: one pipeline-refill
            # boundary per block instead of two, and the exp pipeline stays
            # two activations deep
            ctxp = [cx.tile([128, 512], f32, tag="cx", name=f"ctxp{h}")
                    for h in range(2)]
            for kp in range(NKT // 2):
                for h in range(2):
                    if fillers and (h, kp) in fillers:
                        for fn in fillers[(h, kp)]:
                            fn()
                    hs = slice(h * 64, (h + 1) * 64)
                    # two kt's scoresT share one 2-bank psum tile so a
                    # single 1024-wide exp covers both
                    scp = sc.tile([128, 1024], f32, tag="sc", name="scp")
                    probs = ppool.tile([128, 1024], f16, tag="pb",
                                       name="probs")
                    for j in range(2):
                        kt = kp * 2 + j
                        ksl = slice((b * NKT + kt) * 128,
                                    (b * NKT + kt + 1) * 128)
                        nc.tensor.matmul(scp[:, j * 512:(j + 1) * 512],
                                         kTr[hs, ksl], qTr[hs, qsl],
                                         start=True, stop=True)
                    nc.scalar.activation(probs[:], scp[:], EXP, scale=0.125)
                    for j in range(2):
                        kt = kp * 2 + j
                        nc.tensor.matmul(
                            ctxp[h][:], aug[:, b * NKT + kt, h, :],
                            probs[:, j * 512:(j + 1) * 512],
                            start=(kt == 0), stop=(kt == NKT - 1))
            for h in range(2):
                vrows = slice(0, 64) if h == 0 else slice(64, 128)
                srow = 64 if h == 0 else 0  # psum row holding the exp sums
                # broadcast the sums row across the 64 ctx partitions via a
                # PE ones-matmul, reciprocal on DVE, then normalize.
                srt = npool.tile([128, 512], f16, tag="srt")
                nc.vector.tensor_copy(srt[srow:srow + 1, :],
                                      ctxp[h][srow:srow + 1, :])
                # broadcast the sums row to all 128 psum partitions (a column
                # tile offset of 64 silently misfires for 16-bit matmuls)
                bcp = pj.tile([128, 512], f32, tag="pj", name="bcp")
                nc.tensor.matmul(bcp[:, :], ones[srow:srow + 1, :],
                                 srt[srow:srow + 1, :], start=True, stop=True)
                bcs = npool.tile([128, 512], f32, tag="bcs")
                # full-tile: the custom DVE op misbehaves at base partition 64
                nc.vector.reciprocal_approx_fast(bcs[:, :], bcp[:, :])
                nc.vector.tensor_mul(ctxT[b][vrows, osl], ctxp[h][vrows, :],
                                     bcs[vrows, :])

        def out_proj_tiles(b, tts, copy_eng=None, dma_eng=None):
            for tt in tts:
                ost = opool.tile([128, D], f16, tag="ost", name="ost")
                for oc in range(2):
                    op = pj.tile([128, 512], f32, tag="pj", name="op")
                    nc.tensor.matmul(op[:], ctxT[b][:, tt * 128:(tt + 1) * 128],
                                     wor[:, oc * 512:(oc + 1) * 512],
                                     start=True, stop=True)
                    if copy_eng is None:
                        nc.vector.tensor_copy(ost[:, oc * 512:(oc + 1) * 512],
                                              op[:])
                    else:
                        copy_eng(ost[:, oc * 512:(oc + 1) * 512], op[:])
                (dma_eng or nc.gpsimd).dma_start(
                    out_d[b * S + tt * 128:b * S + (tt + 1) * 128, :], ost[:])

        def scalar_copy(dst, src):
            nc.scalar.activation(dst, src, mybir.ActivationFunctionType.Copy)

        def P(ch, wr, b_t, dst):
            return lambda: proj_one(ch, wr, b_t, dst)

        def V(ch):
            def f():
                proj_one(ch, wvr, bv, vTs)
                transp_chunk(ch)
            return f

        def OP(b, t0, t1):
            return lambda: out_proj_tiles(b, range(t0, t1))

        # Emission order = scheduler priority hint. Projections are emitted
        # as fillers inside the attention kp loops: scores for kp-pair j only
        # need the k/q/v chunks emitted before it, so attention(0) starts
        # after a single chunk and the batch transition self-paces.
        for ch in range(4):
            load_chunk(ch)
        for ch in range(4, NCH):
            load_chunk(ch)
        proj_one(0, wkr, bk, kTr)
        proj_one(0, wqr, bq, qTr)
        proj_one(0, wvr, bv, vTs)
        transp_chunk(0)
        proj_one(1, wkr, bk, kTr)
        proj_one(1, wvr, bv, vTs)
        transp_chunk(1)
        attention_qc(0, 0, fillers={
            (0, 0): [P(2, wkr, bk, kTr), V(2)],
            (0, 2): [P(3, wkr, bk, kTr)],
            (0, 4): [V(3)],
            (1, 0): [P(1, wqr, bq, qTr)]})
        attention_qc(0, 1, fillers={
            (0, 0): [P(2, wqr, bq, qTr)],
            (0, 2): [OP(0, 0, 1)], (0, 6): [OP(0, 1, 2)],
            (1, 2): [OP(0, 2, 3)], (1, 6): [OP(0, 3, 4)]})
        attention_qc(0, 2, fillers={
            (0, 0): [P(3, wqr, bq, qTr)], (0, 2): [P(4, wkr, bk, kTr)],
            (0, 6): [OP(0, 4, 5)],
            (1, 2): [OP(0, 5, 6)], (1, 6): [OP(0, 6, 7)]})
        attention_qc(0, 3, fillers={
            (0, 0): [P(5, wkr, bk, kTr)], (0, 2): [P(6, wkr, bk, kTr)],
            (0, 6): [OP(0, 7, 8)],
            (1, 2): [OP(0, 8, 9)], (1, 6): [OP(0, 9, 10)]})
        attention_qc(1, 0, fillers={
            (0, 0): [V(4), P(4, wqr, bq, qTr)],
            (0, 2): [P(7, wkr, bk, kTr), V(5)],
            (0, 4): [V(6)],
            (0, 6): [V(7)],
            (1, 0): [P(5, wqr, bq, qTr)],
            (1, 2): [OP(0, 10, 11)], (1, 4): [OP(0, 11, 12)],
            (1, 6): [OP(0, 12, 13)]})
        attention_qc(1, 1, fillers={
            (0, 0): [P(6, wqr, bq, qTr)],
            (0, 2): [OP(0, 13, 14)], (0, 4): [OP(0, 14, 15)],
            (0, 6): [OP(0, 15, 16)],
            (1, 2): [OP(1, 0, 1)], (1, 4): [OP(1, 1, 2)],
            (1, 6): [OP(1, 2, 3)]})
        attention_qc(1, 2, fillers={
            (0, 0): [P(7, wqr, bq, qTr)],
            (0, 2): [OP(1, 3, 4)], (0, 4): [OP(1, 4, 5)],
            (0, 6): [OP(1, 5, 6)],
            (1, 2): [OP(1, 6, 7)], (1, 6): [OP(1, 7, 8)]})
        attention_qc(1, 3, fillers={
            (0, 2): [OP(1, 8, 9)], (0, 4): [OP(1, 9, 10)],
            (0, 6): [OP(1, 10, 11)], (1, 0): [OP(1, 11, 12)]})
        # ScalarE is done with exp by now - run the tail as two independent
        # copy+DMA chains (ScalarE->Act HWDGE and DVE->SP HWDGE) in parallel
        out_proj_tiles(1, range(12, 13), copy_eng=scalar_copy,
                       dma_eng=nc.scalar)
        out_proj_tiles(1, range(13, 14), dma_eng=nc.sync)
        out_proj_tiles(1, range(14, 15), copy_eng=scalar_copy,
                       dma_eng=nc.scalar)
        out_proj_tiles(1, range(15, 16), dma_eng=nc.sync)
        ctx.close()

    nc.compile()
    return nc


_NC = None


def _in_maps(inputs, Wq, bq, Wk, bk, Wv, bv, Wo, bo):
    x = np.ascontiguousarray(np.asarray(inputs, dtype=np.float32).reshape(T, D))
    xT = np.ascontiguousarray(x.T.astype(np.float16))
    Wq = np.asarray(Wq, dtype=np.float32)
    Wk = np.asarray(Wk, dtype=np.float32)
    Wv = np.asarray(Wv, dtype=np.float32)
    Wo = np.asarray(Wo, dtype=np.float32)

    in_maps = []
    for c in range(8):
        sl = slice(c * DC, (c + 1) * DC)
        in_maps.append({
            "xT": xT,
            "wqT": np.ascontiguousarray(Wq[sl].T.astype(np.float16)),
            "wkT": np.ascontiguousarray(Wk[sl].T.astype(np.float16)),
            "wvT": np.ascontiguousarray(Wv[sl].T.astype(np.float16)),
            "woT": np.ascontiguousarray(Wo[:, sl].T.astype(np.float16)),
            "bq": np.ascontiguousarray(np.asarray(bq, np.float32)[sl][:, None]),
            "bk": np.ascontiguousarray(np.asarray(bk, np.float32)[sl][:, None]),
            "bv": np.ascontiguousarray(np.asarray(bv, np.float32)[sl][:, None]),
        })
    return in_maps


def kernel(inputs, Wq, bq, Wk, bk, Wv, bv, Wo, bo):
    global _NC
    if _NC is None:
        _NC = _build()

    in_maps = _in_maps(inputs, Wq, bq, Wk, bk, Wv, bv, Wo, bo)
    res = bass_utils.run_bass_kernel_spmd(_NC, in_maps, core_ids=list(range(8)))
    out = res.results[0]["out"].astype(np.float32)
    for r in res.results[1:]:
        out += r["out"].astype(np.float32)
    out += np.asarray(bo, dtype=np.float32)[None, :]
    return out.reshape(B, S, D)


# revision 30
# speedup vs baseline: 1.0878x; 1.0878x over previous
"""Multi-head attention forward (B=2, S=2048, D=1024, H=16) on 8 Trainium2
NeuronCores, tensor-parallel over heads (2 heads per core).

v3: fp16 datapath + XBAR DMA transpose for V.
  - All matmul operands fp16 (same 1 cyc/row PE stream rate as f32r, half the
    SBUF/PSUM/DMA traffic, 2x DVE modes); PSUM accumulation stays fp32.
  - qT/kT/vT projections from a host-pretransposed fp16 xT; bias add on DVE
    while copying PSUM->SBUF fp16.
  - V transposed into the per-k-tile augmented operand via the DMA XBAR
    (dma_start transpose=True on the SP HWDGE queue) instead of PE matmul
    transposes - frees PE cycles and the DVE aug-fill copies.
  - scoresT[k, q] = kT_h.T @ qT_h per (batch, head); exp via ScalarE with the
    1/sqrt(64) folded into the activation scale, probs written as fp16.
  - ctxT accumulated over k-tiles with an augmented V (ones column) so the
    softmax denominators fall out of the same matmuls for free.
  - normalize: broadcast the sums row with a PE ones-matmul, fast reciprocal
    on DVE, multiply into fp16 ctxT; output projection per 128-token tile,
    fp16 partials DMA'd out; host sums the 8 partial outputs + bo in fp32.
  - loop order: qc outer / head inner, out_proj(1) emitted per qc so the
    final output DMAs overlap the attention tail.
"""
import sys
import os

sys.path.insert(0, '/opt/trn_rl_repo')

import numpy as np
import concourse.bass as bass
import concourse.mybir as mybir
import concourse.tile as tile
from concourse import bacc, bass_utils
import contextlib

f32 = mybir.dt.float32
f16 = mybir.dt.float16
EXP = mybir.ActivationFunctionType.Exp

B, S, D, H, HD = 2, 2048, 1024, 16, 64
T = B * S              # 4096 tokens
DC = 128               # dims per core (2 heads)
KT = 8                 # feature k-tiles (D / 128)
NCH = 8                # projection chunks of 512 tokens
NKT = 16               # k-token tiles per batch (S / 128)
NQC = 4                # q chunks of 512 per (b, h)


def _build():
    nc = bacc.Bacc("TRN2", target_bir_lowering=False, debug=False)
    xT_d = nc.dram_tensor("xT", [D, T], f16, kind="ExternalInput").ap()
    wqT_d = nc.dram_tensor("wqT", [D, DC], f16, kind="ExternalInput").ap()
    wkT_d = nc.dram_tensor("wkT", [D, DC], f16, kind="ExternalInput").ap()
    wvT_d = nc.dram_tensor("wvT", [D, DC], f16, kind="ExternalInput").ap()
    woT_d = nc.dram_tensor("woT", [DC, D], f16, kind="ExternalInput").ap()
    bq_d = nc.dram_tensor("bq", [DC, 1], f32, kind="ExternalInput").ap()
    bk_d = nc.dram_tensor("bk", [DC, 1], f32, kind="ExternalInput").ap()
    bv_d = nc.dram_tensor("bv", [DC, 1], f32, kind="ExternalInput").ap()
    out_d = nc.dram_tensor("out", [T, D], f16, kind="ExternalOutput").ap()

    xT_ap = xT_d.rearrange("(kt p) t -> p kt t", p=128)

    with tile.TileContext(nc) as tc:
        ctx = contextlib.ExitStack()
        cpool = ctx.enter_context(tc.tile_pool(name="cpool", bufs=1))
        xpool = ctx.enter_context(tc.tile_pool(name="xpool", bufs=4))
        ppool = ctx.enter_context(tc.tile_pool(name="ppool", bufs=8))
        npool = ctx.enter_context(tc.tile_pool(name="npool", bufs=2))
        opool = ctx.enter_context(tc.tile_pool(name="opool", bufs=3))
        pj = ctx.enter_context(tc.tile_pool(name="pj", bufs=2, space="PSUM"))
        sc = ctx.enter_context(tc.tile_pool(name="sc", bufs=2, space="PSUM"))
        cx = ctx.enter_context(tc.tile_pool(name="cx", bufs=2, space="PSUM"))

        # ---- constants / persistent tiles ----
        wqr = cpool.tile([128, KT, DC], f16, tag="wqr")
        wkr = cpool.tile([128, KT, DC], f16, tag="wkr")
        wvr = cpool.tile([128, KT, DC], f16, tag="wvr")
        nc.gpsimd.dma_start(wqr[:], wqT_d.rearrange("(kt p) m -> p kt m", p=128))
        nc.gpsimd.dma_start(wkr[:], wkT_d.rearrange("(kt p) m -> p kt m", p=128))
        nc.gpsimd.dma_start(wvr[:], wvT_d.rearrange("(kt p) m -> p kt m", p=128))
        wor = cpool.tile([128, D], f16, tag="wor")
        nc.gpsimd.dma_start(wor[:], woT_d[:])
        bq = cpool.tile([DC, 1], f32, tag="bq")
        bk = cpool.tile([DC, 1], f32, tag="bk")
        bv = cpool.tile([DC, 1], f32, tag="bv")
        nc.sync.dma_start(bq[:], bq_d[:])
        nc.sync.dma_start(bk[:], bk_d[:])
        nc.sync.dma_start(bv[:], bv_d[:])

        ones = cpool.tile([128, 128], f16, tag="ones")
        nc.vector.memset(ones[:], 1.0)

        # aug[p, tt, h, :]: per k-token-tile per head augmented V operand.
        # h0: v dims at cols 0..63, ones col 64 -> ctx rows 0..63, sums row 64
        # h1: v dims at cols 64..127, ones col 0 -> ctx rows 64..127, sums row 0
        aug = cpool.tile([128, B * NKT, 2, 128], f16, tag="aug")
        nc.vector.memset(aug[:], 0.0)
        nc.vector.tensor_copy(aug[:, :, 0, 64:65], ones[:, 0:B * NKT])
        nc.vector.tensor_copy(aug[:, :, 1, 0:1], ones[:, 0:B * NKT])

        qTr = cpool.tile([128, T], f16, tag="qTr")
        kTr = cpool.tile([128, T], f16, tag="kTr")
        vTs = cpool.tile([128, T], f16, tag="vTs")
        ctxT = [cpool.tile([128, S], f16, tag=f"ctxT{b}", name=f"ctxT{b}")
                for b in range(B)]

        # ---- phase 1: projections + v transposes ----
        _xtiles = {}

        def load_chunk(ch):
            csl = slice(ch * 512, (ch + 1) * 512)
            xTr = xpool.tile([128, KT, 512], f16, tag="xTr")
            if ch == 0:
                # split the first chunk's load per feature tile so the first
                # projection matmuls can start as soon as f=0 lands
                for f in range(KT):
                    nc.sync.dma_start(xTr[:, f], xT_ap[:, f, csl])
            else:
                # batch-1 chunks go on the gpsimd SWDGE queue: the SP queue
                # is saturated with XBAR transposes during attention(0)
                eng = nc.sync if ch < 4 else nc.gpsimd
                eng.dma_start(xTr[:], xT_ap[:, :, csl])
            _xtiles[ch] = xTr

        def proj_one(ch, wr, b_t, dst):
            csl = slice(ch * 512, (ch + 1) * 512)
            xTr = _xtiles[ch]
            pp = pj.tile([128, 512], f32, tag="pj")
            for f in range(KT):
                nc.tensor.matmul(pp[:], wr[:, f], xTr[:, f],
                                 start=(f == 0), stop=(f == KT - 1))
            nc.vector.tensor_scalar_add(dst[:, csl], pp[:], b_t[:])

        def transp_chunk(ch):
            # transpose v into the aug tiles via the DMA XBAR: one transfer
            # per head covers this chunk's 4 k-token tiles
            csl = slice(ch * 512, (ch + 1) * 512)
            for h in range(2):
                nc.sync.dma_start(
                    aug[:, ch * 4:(ch + 1) * 4, h, h * 64:(h + 1) * 64],
                    vTs[h * 64:(h + 1) * 64, csl], transpose=True)

        # ---- phase 2/3: attention + output projection ----
        def attention_qc(b, qc, fillers=None):
            qsl = slice(b * S + qc * 512, b * S + (qc + 1) * 512)
            osl = slice(qc * 512, (qc + 1) * 512)
            for h in range(2):
                hs = slice(h * 64, (h + 1) * 64)
                vrows = slice(0, 64) if h == 0 else slice(64, 128)
                srow = 64 if h == 0 else 0  # psum row holding the exp sums
                ctxp = cx.tile([128, 512], f32, tag="cx", name="ctxp")
                # process k-tiles in pairs: two kt's scoresT share one
                # 2-bank psum tile so a single 1024-wide exp covers both
                for kp in range(NKT // 2):
                    if fillers and (h, kp) in fillers:
                        for fn in fillers[(h, kp)]:
                            fn()
                    scp = sc.tile([128, 1024], f32, tag="sc", name="scp")
                    probs = ppool.tile([128, 1024], f16, tag="pb", name="probs")
                    for j in range(2):
                        kt = kp * 2 + j
                        ksl = slice((b * NKT + kt) * 128,
                                    (b * NKT + kt + 1) * 128)
                        nc.tensor.matmul(scp[:, j * 512:(j + 1) * 512],
                                         kTr[hs, ksl], qTr[hs, qsl],
                                         start=True, stop=True)
                    nc.scalar.activation(probs[:], scp[:], EXP, scale=0.125)
                    for j in range(2):
                        kt = kp * 2 + j
                        nc.tensor.matmul(
                            ctxp[:], aug[:, b * NKT + kt, h, :],
                            probs[:, j * 512:(j + 1) * 512],
                            start=(kt == 0), stop=(kt == NKT - 1))
                # broadcast the sums row across the 64 ctx partitions via a
                # PE ones-matmul, reciprocal on DVE, then normalize.
                srt = npool.tile([128, 512], f16, tag="srt")
                nc.vector.tensor_copy(srt[srow:srow + 1, :],
                                      ctxp[srow:srow + 1, :])
                # broadcast the sums row to all 128 psum partitions (a column
                # tile offset of 64 silently misfires for 16-bit matmuls)
                bcp = cx.tile([128, 512], f32, tag="cx", name="bcp")
                nc.tensor.matmul(bcp[:, :], ones[srow:srow + 1, :],
                                 srt[srow:srow + 1, :], start=True, stop=True)
                bcs = npool.tile([128, 512], f32, tag="bcs")
                # full-tile: the custom DVE op misbehaves at base partition 64
                nc.vector.reciprocal_approx_fast(bcs[:, :], bcp[:, :])
                nc.vector.tensor_mul(ctxT[b][vrows, osl], ctxp[vrows, :],
                                     bcs[vrows, :])

        def out_proj_tiles(b, tts, copy_eng=None, dma_eng=None):
            for tt in tts:
                ost = opool.tile([128, D], f16, tag="ost", name="ost")
                for oc in range(2):
                    op = pj.tile([128, 512], f32, tag="pj", name="op")
                    nc.tensor.matmul(op[:], ctxT[b][:, tt * 128:(tt + 1) * 128],
                                     wor[:, oc * 512:(oc + 1) * 512],
                                     start=True, stop=True)
                    if copy_eng is None:
                        nc.vector.tensor_copy(ost[:, oc * 512:(oc + 1) * 512],
                                              op[:])
                    else:
                        copy_eng(ost[:, oc * 512:(oc + 1) * 512], op[:])
                (dma_eng or nc.gpsimd).dma_start(
                    out_d[b * S + tt * 128:b * S + (tt + 1) * 128, :], ost[:])

        def scalar_copy(dst, src):
            nc.scalar.activation(dst, src, mybir.ActivationFunctionType.Copy)

        def P(ch, wr, b_t, dst):
            return lambda: proj_one(ch, wr, b_t, dst)

        def V(ch):
            def f():
                proj_one(ch, wvr, bv, vTs)
                transp_chunk(ch)
            return f

        def OP(b, t0, t1):
            return lambda: out_proj_tiles(b, range(t0, t1))

        # Emission order = scheduler priority hint. Projections are emitted
        # as fillers inside the attention kp loops: scores for kp-pair j only
        # need the k/q/v chunks emitted before it, so attention(0) starts
        # after a single chunk and the batch transition self-paces.
        for ch in range(4):
            load_chunk(ch)
        for ch in range(4, NCH):
            load_chunk(ch)
        proj_one(0, wkr, bk, kTr)
        proj_one(0, wqr, bq, qTr)
        proj_one(0, wvr, bv, vTs)
        transp_chunk(0)
        proj_one(1, wkr, bk, kTr)
        proj_one(1, wvr, bv, vTs)
        transp_chunk(1)
        attention_qc(0, 0, fillers={
            (0, 0): [P(2, wkr, bk, kTr), V(2)],
            (0, 2): [P(3, wkr, bk, kTr)],
            (0, 4): [V(3)],
            (1, 0): [P(1, wqr, bq, qTr)]})
        attention_qc(0, 1, fillers={
            (0, 0): [P(2, wqr, bq, qTr)],
            (0, 2): [OP(0, 0, 1)], (0, 6): [OP(0, 1, 2)],
            (1, 2): [OP(0, 2, 3)], (1, 6): [OP(0, 3, 4)]})
        attention_qc(0, 2, fillers={
            (0, 0): [P(3, wqr, bq, qTr)], (0, 2): [P(4, wkr, bk, kTr)],
            (0, 6): [OP(0, 4, 5)],
            (1, 2): [OP(0, 5, 6)], (1, 6): [OP(0, 6, 7)]})
        attention_qc(0, 3, fillers={
            (0, 0): [P(5, wkr, bk, kTr)], (0, 2): [P(6, wkr, bk, kTr)],
            (0, 6): [OP(0, 7, 8)],
            (1, 2): [OP(0, 8, 9)], (1, 6): [OP(0, 9, 10)]})
        attention_qc(1, 0, fillers={
            (0, 0): [V(4), P(4, wqr, bq, qTr)],
            (0, 2): [P(7, wkr, bk, kTr), V(5)],
            (0, 4): [V(6)],
            (0, 6): [V(7)],
            (1, 0): [P(5, wqr, bq, qTr)],
            (1, 2): [OP(0, 10, 11)], (1, 4): [OP(0, 11, 12)],
            (1, 6): [OP(0, 12, 13)]})
        attention_qc(1, 1, fillers={
            (0, 0): [P(6, wqr, bq, qTr)],
            (0, 2): [OP(0, 13, 14)], (0, 4): [OP(0, 14, 15)],
            (0, 6): [OP(0, 15, 16)],
            (1, 2): [OP(1, 0, 1)], (1, 4): [OP(1, 1, 2)],
            (1, 6): [OP(1, 2, 3)]})
        attention_qc(1, 2, fillers={
            (0, 0): [P(7, wqr, bq, qTr)],
            (0, 2): [OP(1, 3, 4)], (0, 4): [OP(1, 4, 5)],
            (0, 6): [OP(1, 5, 6)],
            (1, 2): [OP(1, 6, 7)], (1, 6): [OP(1, 7, 8)]})
        attention_qc(1, 3, fillers={
            (0, 2): [OP(1, 8, 9)], (0, 4): [OP(1, 9, 10)],
            (0, 6): [OP(1, 10, 11)], (1, 0): [OP(1, 11, 12)]})
        # ScalarE is done with exp by now - run the tail as two independent
        # copy+DMA chains (ScalarE->Act HWDGE and DVE->SP HWDGE) in parallel
        out_proj_tiles(1, range(12, 13), copy_eng=scalar_copy,
                       dma_eng=nc.scalar)
        out_proj_tiles(1, range(13, 14), dma_eng=nc.sync)
        out_proj_tiles(1, range(14, 15), copy_eng=scalar_copy,
                       dma_eng=nc.scalar)
        out_proj_tiles(1, range(15, 16), dma_eng=nc.sync)
        ctx.close()

    nc.compile()
    return nc


_NC = None


def _in_maps(inputs, Wq, bq, Wk, bk, Wv, bv, Wo, bo):
    x = np.ascontiguousarray(np.asarray(inputs, dtype=np.float32).reshape(T, D))
    xT = np.ascontiguousarray(x.T.astype(np.float16))
    Wq = np.asarray(Wq, dtype=np.float32)
    Wk = np.asarray(Wk, dtype=np.float32)
    Wv = np.asarray(Wv, dtype=np.float32)
    Wo = np.asarray(Wo, dtype=np.float32)

    in_maps = []
    for c in range(8):
        sl = slice(c * DC, (c + 1) * DC)
        in_maps.append({
            "xT": xT,
            "wqT": np.ascontiguousarray(Wq[sl].T.astype(np.float16)),
            "wkT": np.ascontiguousarray(Wk[sl].T.astype(np.float16)),
            "wvT": np.ascontiguousarray(Wv[sl].T.astype(np.float16)),
            "woT": np.ascontiguousarray(Wo[:, sl].T.astype(np.float16)),
            "bq": np.ascontiguousarray(np.asarray(bq, np.float32)[sl][:, None]),
            "bk": np.ascontiguousarray(np.asarray(bk, np.float32)[sl][:, None]),
            "bv": np.ascontiguousarray(np.asarray(bv, np.float32)[sl][:, None]),
        })
    return in_maps


def kernel(inputs, Wq, bq, Wk, bk, Wv, bv, Wo, bo):
    global _NC
    if _NC is None:
        _NC = _build()

    in_maps = _in_maps(inputs, Wq, bq, Wk, bk, Wv, bv, Wo, bo)
    res = bass_utils.run_bass_kernel_spmd(_NC, in_maps, core_ids=list(range(8)))
    out = res.results[0]["out"].astype(np.float32)
    for r in res.results[1:]:
        out += r["out"].astype(np.float32)
    out += np.asarray(bo, dtype=np.float32)[None, :]
    return out.reshape(B, S, D)
